# revision 10
# baseline (speedup 1.0000x reference)
"""GAT (2-layer, CORA-style) on 8 Trainium2 NeuronCores.

Strategy: nodes dst-sharded across 8 cores (profile-sorted for padding).
  Phase 1: h = x@W1 in bf16 (+ attention scores), packed fp16 per-node
           table rows [h(64) | a_src(8)] -> DRAM, AllGather, then a
           restride DMA lays the table out at 256B row stride in 4
           windows of 25000 rows (+1 leading sentinel row per window)
           so gather indices fit int16.
  Edge phases: per group of 2 tiles (256 dsts), batched dma_gather ops
           (one per tile x window, ~2-3K rows each) fetch 144B rows at
           256B stride; segment-softmax with contiguous tree reductions;
           per-dst slot columns are padded per-group-uniform via
           sentinel rows (score -60 -> exp ~ 0).
  Phase 2: tiny fused matmul [W2 | W2@as2 | W2@ad2], second AllGather +
           restride (16B rows at 256B stride), second edge phase reusing
           the SAME index lists, log_softmax, output shard.
"""

import sys

sys.path.insert(0, "/opt/trn_rl_repo")

import numpy as np
import ml_dtypes

BF16 = ml_dtypes.bfloat16

N, E, F_IN = 100000, 3200000, 1433
H1, HID, NCLS = 8, 8, 7
NC = 8
NSH = N // NC
NW = 4
WNODES = 25000  # nodes per window
WROWS = WNODES + 1  # +1 sentinel row at window start
NROWT = NW * WROWS  # 100004
ROW1 = H1 * HID + H1  # 72
ROW2 = NCLS + 1  # 8
C1 = H1 * HID  # 64
VE = 128  # table row stride in f16 elements (256B)
NTG = 2  # tiles per group
RING = 1024  # max idxs per gather op (SWDGE ring capacity)
SCRATCH = 16384


def _win_row(r):
    # permuted-global node id -> strided table row index
    return (r // WNODES) * WROWS + 1 + (r % WNODES)


# --------------------------------------------------------------------------
def host_prep(inputs):
    x = np.asarray(inputs["x"], dtype=np.float32)
    ei = np.asarray(inputs["edge_index"], dtype=np.int64)
    W1 = np.asarray(inputs["W1"], dtype=np.float32)
    as1 = np.asarray(inputs["att_src1"], dtype=np.float32)
    ad1 = np.asarray(inputs["att_dst1"], dtype=np.float32)
    b1 = np.asarray(inputs["b1"], dtype=np.float32)
    W2 = np.asarray(inputs["W2"], dtype=np.float32)
    as2 = np.asarray(inputs["att_src2"], dtype=np.float32)
    ad2 = np.asarray(inputs["att_dst2"], dtype=np.float32)
    b2 = np.asarray(inputs["b2"], dtype=np.float32)

    loops = np.arange(N, dtype=np.int64)
    src_all = np.concatenate([ei[0], loops])
    dst_all = np.concatenate([ei[1], loops])

    # per-dst per-window counts of srcs (window by ORIGINAL src id is wrong:
    # windows are over PERMUTED ids; permutation is per-core independent, so
    # compute profiles over a provisional id = original (windows align with
    # final perm closely enough for sorting; exact cols computed later).
    # -> To keep it exact, do a 2-pass: first a degree-based permutation of
    # SRC ids is identity (srcs use the same gid map as dsts). We build the
    # permutation from profiles over original src windows, then recompute
    # everything with the actual gid.
    prof0 = np.zeros((N, NW), np.int32)
    np.add.at(prof0, (dst_all, np.minimum(src_all // WNODES, NW - 1)), 1)

    gid = np.empty(N, dtype=np.int64)
    perm_nodes = []
    for c in range(NC):
        lo = c * NSH
        p = prof0[lo : lo + NSH]
        # sort by the padding objective: a tile's cost is max over its rows
        # of max-per-window count; group rows with similar maxima together
        mw = p.max(axis=1)
        order = np.lexsort((-p.sum(axis=1), -mw))
        perm_nodes.append(lo + order)
        pos = np.empty(NSH, dtype=np.int64)
        pos[order] = np.arange(NSH)
        gid[lo : lo + NSH] = lo + pos

    pd = gid[dst_all]
    gs = gid[src_all].astype(np.int64)
    eorder = np.argsort(pd, kind="stable")
    gss = gs[eorder]
    counts = np.bincount(pd, minlength=N)
    row_start = np.zeros(N + 1, np.int64)
    np.cumsum(counts, out=row_start[1:])

    NT = (NSH + 127) // 128
    # per (core, tile-row p, window w) sorted src lists
    # cols[t][w] = max over cores,rows of per-window count
    cols = np.zeros((NT, NW), np.int64)
    wof_s = gss // WNODES  # window of each (dst-sorted) edge's src
    for c in range(NC):
        base = c * NSH
        cw = np.zeros((NSH, NW), np.int32)
        np.add.at(
            cw,
            (
                np.repeat(np.arange(NSH), np.diff(row_start[base : base + NSH + 1])),
                wof_s[row_start[base] : row_start[base + NSH]],
            ),
            1,
        )
        for t in range(NT):
            blk = cw[t * 128 : (t + 1) * 128]
            cols[t] = np.maximum(cols[t], blk.max(axis=0))

    # groups of NTG tiles with uniform per-(t,w) col count C_g (even)
    groups = []
    for g0 in range(0, NT, NTG):
        tl = list(range(g0, min(g0 + NTG, NT)))
        C = int(max(cols[t].max() for t in tl))
        C = max(2, C + (C & 1))
        groups.append(dict(tiles=tl, C=C))

    # build idx lists (shared schedule; per-core values)
    # G layout per group: [tile][window][C] columns, each column = 128 slots
    # op = (tile, window, col-subrange) with ni = 128*ncols <= RING
    idx_parts = [[] for _ in range(NC)]
    op_sched = []  # per group: list of (gcol0, ncols, w, ni, idx_off)
    idx_off = 0
    for gi, grp in enumerate(groups):
        C = grp["C"]
        ops = []
        for ti, t in enumerate(grp["tiles"]):
            for w in range(NW):
                ncols_tot = C
                csub0 = 0
                while csub0 < ncols_tot:
                    ncols = min(ncols_tot - csub0, RING // 128)
                    ni = 128 * ncols
                    gcol0 = ti * (NW * C) + w * C + csub0
                    ops.append((gcol0, ncols, w, ni, idx_off))
                    idx_off += 128 * (ni // 16)  # int16 elements (wrapped layout)
                    csub0 += ncols
        grp["ops"] = ops
        op_sched.append(ops)
    TOTIDX = idx_off

    # fill per-core idx data
    for c in range(NC):
        base = c * NSH
        # per row: sorted srcs split by window (window-local strided row ids)
        row_lists = []
        for p in range(NSH):
            seg = np.sort(gss[row_start[base + p] : row_start[base + p + 1]])
            wsplit = []
            for w in range(NW):
                lo = np.searchsorted(seg, w * WNODES)
                hi = np.searchsorted(seg, (w + 1) * WNODES)
                wsplit.append(1 + (seg[lo:hi] - w * WNODES))
            row_lists.append(wsplit)
        arr = np.zeros(TOTIDX, dtype=np.int16)
        for gi, grp in enumerate(groups):
            C = grp["C"]
            for gcol0, ncols, w, ni, off in grp["ops"]:
                ti = gcol0 // (NW * C)
                csub0 = gcol0 - ti * (NW * C) - w * C
                t = grp["tiles"][ti]
                # idx list: column-major: i = (c_loc*128 + p)
                blk = np.zeros((ncols, 128), dtype=np.int16)
                for p in range(min(128, NSH - t * 128)):
                    lst = row_lists[t * 128 + p][w]
                    lo = csub0
                    hi = min(len(lst), csub0 + ncols)
                    if hi > lo:
                        blk[0 : hi - lo, p] = lst[lo:hi]
                flat = blk.reshape(-1)  # i = c_loc*128+p
                # wrapped [128, ni/16]: entry i at [i%16, i//16], replicated x8
                wrap = np.zeros((128, ni // 16), dtype=np.int16)
                w16 = flat.reshape(ni // 16, 16).T
                for rg in range(8):
                    wrap[rg * 16 : (rg + 1) * 16, :] = w16
                arr[off : off + 128 * (ni // 16)] = wrap.reshape(-1)
        idx_parts[c] = arr

    # x shards (bf16, transposed)
    xts = [np.ascontiguousarray(x[perm_nodes[c]].T).astype(BF16) for c in range(NC)]

    attm = np.zeros((C1, 2 * H1), dtype=np.float32)
    for h in range(H1):
        attm[h * HID : (h + 1) * HID, h] = as1[h]
        attm[h * HID : (h + 1) * HID, H1 + h] = ad1[h]
    m2 = np.zeros((HID, NCLS + 2), dtype=np.float32)
    m2[:, :NCLS] = W2
    m2[:, NCLS] = W2 @ as2[0]
    m2[:, NCLS + 1] = W2 @ ad2[0]
    b1r = np.tile(b1[None, :], (128, 1)).astype(np.float32)
    b2r = np.tile(b2[None, :], (128, 1)).astype(np.float32)
    # sentinel rows (one per window, at strided row w*WROWS)
    sent1 = np.zeros((NW, ROW1), dtype=np.float16)
    sent1[:, C1:] = -60.0
    sent2 = np.zeros((NW, ROW2), dtype=np.float16)
    sent2[:, NCLS] = -60.0
    ident = np.eye(128, dtype=np.float32)

    meta = dict(NT=NT, groups=groups, TOTIDX=TOTIDX)
    in_maps = []
    for c in range(NC):
        in_maps.append(
            dict(
                xt=xts[c],
                w1=W1.astype(BF16),
                attm=attm,
                m2=m2.astype(BF16),
                b1r=b1r,
                b2r=b2r,
                sent1=sent1,
                sent2=sent2,
                ident=ident,
                idx=idx_parts[c],
            )
        )
    return meta, in_maps, perm_nodes


# --------------------------------------------------------------------------
def build_program(meta):
    from concourse import bacc, bass, tile, mybir
    from concourse.mybir import InstDMAGatherAnt

    NT = meta["NT"]
    groups = meta["groups"]
    TOTIDX = meta["TOTIDX"]
    f32, f16, i32 = mybir.dt.float32, mybir.dt.float16, mybir.dt.int32
    i16, bf16 = mybir.dt.int16, mybir.dt.bfloat16
    AX = mybir.AxisListType.X
    OP = mybir.AluOpType
    AF = mybir.ActivationFunctionType

    nc = bacc.Bacc(
        "TRN2",
        target_bir_lowering=False,
        debug=False,
        num_devices=NC,
        dynamic_dma_scratch_size=SCRATCH,
    )

    xt = nc.dram_tensor("xt", [F_IN, NSH], bf16, kind="ExternalInput").ap()
    w1 = nc.dram_tensor("w1", [F_IN, C1], bf16, kind="ExternalInput").ap()
    attm = nc.dram_tensor("attm", [C1, 2 * H1], f32, kind="ExternalInput").ap()
    m2 = nc.dram_tensor("m2", [HID, NCLS + 2], bf16, kind="ExternalInput").ap()
    b1r = nc.dram_tensor("b1r", [128, HID], f32, kind="ExternalInput").ap()
    b2r = nc.dram_tensor("b2r", [128, NCLS], f32, kind="ExternalInput").ap()
    sent1 = nc.dram_tensor("sent1", [NW, ROW1], f16, kind="ExternalInput").ap()
    sent2 = nc.dram_tensor("sent2", [NW, ROW2], f16, kind="ExternalInput").ap()
    ident = nc.dram_tensor("ident", [128, 128], f32, kind="ExternalInput").ap()
    idx_h = nc.dram_tensor("idx", [TOTIDX], i16, kind="ExternalInput")
    out = nc.dram_tensor("out", [NSH, NCLS], f32, kind="ExternalOutput").ap()

    ktl = []
    k0 = 0
    while k0 < F_IN:
        ktl.append((k0, min(128, F_IN - k0)))
        k0 += 128
    chunks = []
    n0 = 0
    while n0 < NSH:
        chunks.append((n0, min(512, NSH - n0)))
        n0 += 512

    def slices_of(c0, csz):
        s0, outl = 0, []
        while s0 < csz:
            ssz = min(128, csz - s0)
            outl.append((c0 + s0, ssz))
            s0 += ssz
        return outl

    def raw_gather(out_ap, in_ap, idxs_ap, num_idxs, elem_size):
        eng = nc.gpsimd
        elem_step = in_ap.ap[0][0]
        stride_bytes = elem_step * mybir.dt.size(in_ap.dtype)
        return eng.add_instruction(
            InstDMAGatherAnt(
                name=f"I-{eng.bass.next_id()}",
                ins=[
                    *eng.lower_ap_dma(in_ap, for_custom_bir_dma=True),
                    eng.lower_ap(idxs_ap),
                    eng.lower_val_access(eng.to_reg(num_idxs)),
                ],
                outs=[eng.lower_ap(out_ap)],
                transpose=False,
                num_idxs=num_idxs,
                elem_size=elem_size,
                stride_bytes_256=stride_bytes // 256,
                gen_mode=0,
                single_packet=True,
                queue_num=0,
                sbuf_tokens_per_rank=0,
                sbuf_free_dim_per_rank=0,
                sbuf_free_dim_pad_per_rank=0,
                sbuf_byte_offset=0,
            )
        )

    with tile.TileContext(nc) as tc:
        with (
            tc.tile_pool(name="dram", bufs=1, space="DRAM") as dpool,
            tc.tile_pool(name="consts", bufs=1) as cpool,
            tc.tile_pool(name="persist", bufs=1) as ppool,
            tc.tile_pool(name="xload", bufs=4) as xpool,
            tc.tile_pool(name="work", bufs=2) as wpool,
            tc.tile_pool(name="scr", bufs=1) as spool,
            tc.tile_pool(name="ps", bufs=2, space="PSUM") as pspool,
        ):
            tb1s = dpool.tile([NSH, ROW1], f16, tag="tb1s")
            tb1c = dpool.tile([N, ROW1], f16, tag="tb1c")
            tb1f = dpool.tile([NROWT, VE], f16, tag="tb1f")
            tb2s = dpool.tile([NSH, ROW2], f16, tag="tb2s")
            tb2c = dpool.tile([N, ROW2], f16, tag="tb2c")
            tb2f = dpool.tile([NROWT, VE], f16, tag="tb2f")

            w1t = []
            for ki, (k0, ks) in enumerate(ktl):
                wt = cpool.tile([ks, C1], bf16, tag=f"w1_{ki}")
                nc.sync.dma_start(wt[:], w1[k0 : k0 + ks, :])
                w1t.append(wt)
            attm_sb = cpool.tile([C1, 2 * H1], f32, tag="attm")
            nc.sync.dma_start(attm_sb[:], attm[:])
            m2_sb = cpool.tile([HID, NCLS + 2], bf16, tag="m2")
            nc.sync.dma_start(m2_sb[:], m2[:])
            b1_sb = cpool.tile([128, HID], f32, tag="b1")
            nc.sync.dma_start(b1_sb[:], b1r[:])
            b2_sb = cpool.tile([128, NCLS], f32, tag="b2")
            nc.sync.dma_start(b2_sb[:], b2r[:])
            id_sb = cpool.tile([128, 128], f32, tag="ident")
            nc.sync.dma_start(id_sb[:], ident[:])
            s1_sb = cpool.tile([NW, ROW1], f16, tag="s1")
            nc.sync.dma_start(s1_sb[:], sent1[:])
            s2_sb = cpool.tile([NW, ROW2], f16, tag="s2")
            nc.sync.dma_start(s2_sb[:], sent2[:])
            nbias = cpool.tile([128, 1], f32, tag="nbias")
            nc.gpsimd.memset(nbias[:], -8.0)
            # sentinel rows into strided tables (row w*WROWS)
            for w in range(NW):
                nc.sync.dma_start(
                    tb1f[w * WROWS : w * WROWS + 1, 0:ROW1], s1_sb[w : w + 1, :]
                )
                nc.sync.dma_start(
                    tb2f[w * WROWS : w * WROWS + 1, 0:ROW2], s2_sb[w : w + 1, :]
                )

            ad1_all = ppool.tile([128, NT * H1], f32, tag="ad1_all")
            ad2_all = ppool.tile([128, NT], f32, tag="ad2_all")
            eluT = ppool.tile([HID, NSH], bf16, tag="eluT")

            # =========================== PHASE 1 ===========================
            for ch, (c0, csz) in enumerate(chunks):
                hp = pspool.tile([C1, csz], f32, tag="mm")
                for ki, (k0, ks) in enumerate(ktl):
                    xtt = xpool.tile([ks, csz], bf16, tag="xt")
                    nc.sync.dma_start(xtt[:], xt[k0 : k0 + ks, c0 : c0 + csz])
                    nc.tensor.matmul(
                        hp[:],
                        w1t[ki][:],
                        xtt[:],
                        start=(ki == 0),
                        stop=(ki == len(ktl) - 1),
                    )
                h_sb = wpool.tile([C1, csz], f32, tag="h_sb")
                nc.vector.tensor_copy(h_sb[:], hp[:])
                sp = pspool.tile([2 * H1, csz], f32, tag="sc")
                nc.tensor.matmul(sp[:], attm_sb[:], h_sb[:], start=True, stop=True)
                sc_sb = wpool.tile([2 * H1, csz], f32, tag="sc_sb")
                nc.vector.tensor_copy(sc_sb[:], sp[:])
                for s0, ssz in slices_of(c0, csz):
                    t_idx = s0 // 128
                    sl = slice(s0 - c0, s0 - c0 + ssz)
                    tp = pspool.tile([ssz, C1], f32, tag="tr")
                    nc.tensor.transpose(tp[:], h_sb[:, sl], id_sb[:C1, :C1])
                    st = pspool.tile([ssz, 2 * H1], f32, tag="tr2")
                    nc.tensor.transpose(st[:], sc_sb[:, sl], id_sb[: 2 * H1, : 2 * H1])
                    row = wpool.tile([ssz, ROW1], f16, tag="row")
                    nc.scalar.activation(row[:, 0:C1], tp[:], AF.Copy)
                    nc.vector.tensor_copy(row[:, C1:ROW1], st[:, 0:H1])
                    nc.vector.tensor_copy(
                        ad1_all[0:ssz, t_idx * H1 : (t_idx + 1) * H1],
                        st[:, H1 : 2 * H1],
                    )
                    nc.sync.dma_start(tb1s[s0 : s0 + ssz, :], row[:])

            nc.gpsimd.collective_compute(
                "AllGather",
                OP.bypass,
                replica_groups=[list(range(NC))],
                ins=[tb1s[:].opt()],
                outs=[tb1c[:].opt()],
            )
            # restride into windowed 256B-stride layout
            for w in range(NW):
                nc.sync.dma_start(
                    tb1f[w * WROWS + 1 : w * WROWS + 1 + WNODES, 0:ROW1],
                    tb1c[w * WNODES : (w + 1) * WNODES, :],
                )

            # =========================== EDGE PHASE 1 ======================
            for gi, grp in enumerate(groups):
                C = grp["C"]
                tl = grp["tiles"]
                ntg = len(tl)
                S = ntg * NW * C  # total columns in group
                G = wpool.tile([128, S * ROW1], f16, tag="G")
                for gcol0, ncols, w, ni, off in grp["ops"]:
                    itb = wpool.tile([128, ni // 16], i16, tag="itb")
                    nc.sync.dma_start(
                        itb[:],
                        bass.AP(idx_h, off, [[ni // 16, 128], [1, ni // 16]]),
                    )
                    raw_gather(
                        G[:, gcol0 * ROW1 : (gcol0 + ncols) * ROW1].rearrange(
                            "p (j v) -> p j v", v=ROW1
                        ),
                        tb1f[w * WROWS : (w + 1) * WROWS, 0:ROW1],
                        itb[:],
                        ni,
                        ROW1,
                    )
                # scores: s = a_src (from G) + a_dst (local)
                G4 = G[:].rearrange("p (t q v) -> p t q v", t=ntg, v=ROW1)
                sT = spool.tile([128, S * H1], f32, tag="sT")
                ad_b = (
                    ad1_all[:, tl[0] * H1 : (tl[0] + ntg) * H1]
                    .rearrange("p (t h) -> p t h", t=ntg)
                    .unsqueeze(2)
                    .broadcast_to([128, ntg, NW * C, H1])
                )
                nc.vector.tensor_tensor(
                    sT[:].rearrange("p (t q h) -> p t q h", t=ntg, h=H1),
                    G4[:, :, :, C1:ROW1],
                    ad_b,
                    OP.add,
                )
                pT = spool.tile([128, S * H1], f16, tag="pT")
                nc.vector.tensor_scalar(pT[:], sT[:], 0.2, None, op0=OP.mult)
                nc.vector.tensor_tensor(sT[:], sT[:], pT[:], OP.max)
                nc.scalar.activation(pT[:], sT[:], AF.Exp, bias=nbias[:])
                # den tree: [128, ntg, NW*C, 8] halve axis 2
                q = NW * C
                dt1 = spool.tile([128, ntg * (q // 2) * H1], f16, tag="dt1")
                dt2 = spool.tile([128, ntg * (q // 4 + 1) * H1], f16, tag="dt2")

                def tree(src_ap, qq, t1, t2, width):
                    # src_ap viewed [128, ntg, qq, width]; returns ap [128, ntg, 1, width]
                    cur = src_ap
                    buf = [t1, t2]
                    bi = 0
                    while qq > 1:
                        half = qq // 2
                        odd = qq - 2 * half
                        dst = buf[bi][:].rearrange(
                            "p (t q w) -> p t q w", t=ntg, w=width
                        )[:, :, 0 : half + odd, :]
                        nc.vector.tensor_tensor(
                            dst[:, :, 0:half, :],
                            cur[:, :, 0:half, :],
                            cur[:, :, half : 2 * half, :],
                            OP.add,
                        )
                        if odd:
                            nc.vector.tensor_copy(
                                dst[:, :, half : half + 1, :],
                                cur[:, :, 2 * half : qq, :],
                            )
                        cur = dst
                        qq = half + odd
                        bi ^= 1
                    return cur

                pT4 = pT[:].rearrange("p (t q h) -> p t q h", t=ntg, h=H1)
                denc = tree(pT4, q, dt1, dt2, H1)
                den = spool.tile([128, ntg * H1], f32, tag="den")
                nc.vector.tensor_copy(
                    den[:].rearrange("p (t h) -> p t h", t=ntg),
                    denc.squeeze(2),
                )
                rr = spool.tile([128, ntg * H1], f32, tag="rr")
                nc.vector.reciprocal(rr[:], den[:])
                nc.vector.tensor_scalar(rr[:], rr[:], 1.0 / H1, None, op0=OP.mult)
                # messages m1 = h * p (in-place into G's h columns)
                Ghv = (
                    G[:]
                    .rearrange("p (s v) -> p s v", v=ROW1)[:, :, 0:C1]
                    .rearrange("p s (h c) -> p s h c", h=H1)
                )
                nc.vector.tensor_tensor(
                    Ghv,
                    Ghv,
                    pT[:]
                    .rearrange("p (s h) -> p s h", h=H1)
                    .unsqueeze(3)
                    .broadcast_to([128, S, H1, HID]),
                    OP.mult,
                )
                mt1 = spool.tile([128, ntg * (q // 2) * C1], f16, tag="mt1")
                mt2 = spool.tile([128, ntg * (q // 4 + 1) * C1], f16, tag="mt2")
                m14 = G[:].rearrange("p (t q v) -> p t q v", t=ntg, v=ROW1)[
                    :, :, :, 0:C1
                ]
                uc = tree(m14, q, mt1, mt2, C1)
                # o1 = u * rr (broadcast over c), then mean over h, +b1, elu
                o1 = spool.tile([128, ntg * C1], f32, tag="o1")
                nc.vector.tensor_tensor(
                    o1[:].rearrange("p (t h c) -> p t h c", t=ntg, c=HID),
                    uc.squeeze(2).rearrange("p t (h c) -> p t h c", c=HID),
                    rr[:]
                    .rearrange("p (t h) -> p t h", t=ntg)
                    .unsqueeze(3)
                    .broadcast_to([128, ntg, H1, HID]),
                    OP.mult,
                )
                om = spool.tile([128, ntg * HID], f32, tag="om")
                nc.vector.tensor_reduce(
                    om[:].rearrange("p (t c) -> p t c", t=ntg),
                    o1[:].rearrange("p (t h c) -> p t c h", t=ntg, c=HID),
                    axis=AX,
                    op=OP.add,
                )
                nc.vector.tensor_tensor(
                    om[:].rearrange("p (t c) -> p t c", t=ntg),
                    om[:].rearrange("p (t c) -> p t c", t=ntg),
                    b1_sb[:, :].unsqueeze(1).broadcast_to([128, ntg, HID]),
                    OP.add,
                )
                mn = spool.tile([128, ntg * HID], f32, tag="mn")
                nc.vector.tensor_scalar(mn[:], om[:], 0.0, None, op0=OP.min)
                nc.scalar.activation(mn[:], mn[:], AF.Exp)
                rl = spool.tile([128, ntg * HID], f32, tag="rl")
                nc.vector.tensor_scalar(rl[:], om[:], 0.0, None, op0=OP.max)
                nc.vector.tensor_scalar(mn[:], mn[:], -1.0, None, op0=OP.add)
                nc.vector.tensor_tensor(rl[:], rl[:], mn[:], OP.add)
                for ti, t in enumerate(tl):
                    P = min(128, NSH - t * 128)
                    ep = pspool.tile([HID, P], f32, tag="sc")
                    nc.tensor.transpose(
                        ep[:], rl[0:P, ti * HID : (ti + 1) * HID], id_sb[:P, :P]
                    )
                    nc.scalar.activation(
                        eluT[:, t * 128 : t * 128 + P], ep[:], AF.Copy
                    )

            # =========================== PHASE 2 ===========================
            for ch, (c0, csz) in enumerate(chunks):
                o2p = pspool.tile([NCLS + 2, csz], f32, tag="mm")
                nc.tensor.matmul(
                    o2p[:], m2_sb[:], eluT[:, c0 : c0 + csz], start=True, stop=True
                )
                o2_sb = wpool.tile([NCLS + 2, csz], f32, tag="o2_sb")
                nc.vector.tensor_copy(o2_sb[:], o2p[:])
                for s0, ssz in slices_of(c0, csz):
                    t_idx = s0 // 128
                    sl = slice(s0 - c0, s0 - c0 + ssz)
                    tp2 = pspool.tile([ssz, NCLS + 2], f32, tag="tr")
                    nc.tensor.transpose(
                        tp2[:], o2_sb[:, sl], id_sb[: NCLS + 2, : NCLS + 2]
                    )
                    row2 = wpool.tile([ssz, ROW2], f16, tag="row2")
                    nc.scalar.activation(row2[:], tp2[:, 0:ROW2], AF.Copy)
                    nc.vector.tensor_copy(
                        ad2_all[0:ssz, t_idx : t_idx + 1], tp2[:, ROW2 : ROW2 + 1]
                    )
                    nc.sync.dma_start(tb2s[s0 : s0 + ssz, :], row2[:])

            nc.gpsimd.collective_compute(
                "AllGather",
                OP.bypass,
                replica_groups=[list(range(NC))],
                ins=[tb2s[:].opt()],
                outs=[tb2c[:].opt()],
            )
            for w in range(NW):
                nc.sync.dma_start(
                    tb2f[w * WROWS + 1 : w * WROWS + 1 + WNODES, 0:ROW2],
                    tb2c[w * WNODES : (w + 1) * WNODES, :],
                )

            # =========================== EDGE PHASE 2 ======================
            for gi, grp in enumerate(groups):
                C = grp["C"]
                tl = grp["tiles"]
                ntg = len(tl)
                S = ntg * NW * C
                q = NW * C
                G2 = wpool.tile([128, S * ROW2], f16, tag="G2")
                for gcol0, ncols, w, ni, off in grp["ops"]:
                    itb = wpool.tile([128, ni // 16], i16, tag="itb")
                    nc.sync.dma_start(
                        itb[:],
                        bass.AP(idx_h, off, [[ni // 16, 128], [1, ni // 16]]),
                    )
                    raw_gather(
                        G2[:, gcol0 * ROW2 : (gcol0 + ncols) * ROW2].rearrange(
                            "p (j v) -> p j v", v=ROW2
                        ),
                        tb2f[w * WROWS : (w + 1) * WROWS, 0:ROW2],
                        itb[:],
                        ni,
                        ROW2,
                    )
                s2 = spool.tile([128, S], f32, tag="s2")
                nc.vector.tensor_tensor(
                    s2[:].rearrange("p (t q) -> p t q", t=ntg),
                    G2[:]
                    .rearrange("p (t q v) -> p t q v", t=ntg, v=ROW2)[:, :, :, NCLS]
                    .squeeze(),
                    ad2_all[:, tl[0] : tl[0] + ntg]
                    .unsqueeze(2)
                    .broadcast_to([128, ntg, q]),
                    OP.add,
                )
                p2 = spool.tile([128, S], f16, tag="p2")
                nc.vector.tensor_scalar(p2[:], s2[:], 0.2, None, op0=OP.mult)
                nc.vector.tensor_tensor(s2[:], s2[:], p2[:], OP.max)
                nc.scalar.activation(p2[:], s2[:], AF.Exp)
                et1 = spool.tile([128, ntg * (q // 2)], f16, tag="et1")
                et2 = spool.tile([128, ntg * (q // 4 + 1)], f16, tag="et2")

                def tree2(src_ap, qq, t1, t2, width):
                    cur = src_ap
                    buf = [t1, t2]
                    bi = 0
                    while qq > 1:
                        half = qq // 2
                        odd = qq - 2 * half
                        dst = buf[bi][:].rearrange(
                            "p (t q w) -> p t q w", t=ntg, w=width
                        )[:, :, 0 : half + odd, :]
                        nc.vector.tensor_tensor(
                            dst[:, :, 0:half, :],
                            cur[:, :, 0:half, :],
                            cur[:, :, half : 2 * half, :],
                            OP.add,
                        )
                        if odd:
                            nc.vector.tensor_copy(
                                dst[:, :, half : half + 1, :],
                                cur[:, :, 2 * half : qq, :],
                            )
                        cur = dst
                        qq = half + odd
                        bi ^= 1
                    return cur

                p24 = p2[:].rearrange("p (t q) -> p t q", t=ntg).unsqueeze(3)
                d2c = tree2(p24, q, et1, et2, 1)
                den2 = spool.tile([128, ntg], f32, tag="den2")
                nc.vector.tensor_copy(den2[:].unsqueeze(2).unsqueeze(3), d2c)
                r2 = spool.tile([128, ntg], f32, tag="r2")
                nc.vector.reciprocal(r2[:], den2[:])
                G2v = G2[:].rearrange("p (s v) -> p s v", v=ROW2)
                nc.vector.tensor_tensor(
                    G2v,
                    G2v,
                    p2[:].unsqueeze(2).broadcast_to([128, S, ROW2]),
                    OP.mult,
                )
                ut1 = spool.tile([128, ntg * (q // 2) * ROW2], f16, tag="ut1")
                ut2 = spool.tile([128, ntg * (q // 4 + 1) * ROW2], f16, tag="ut2")
                m24 = G2[:].rearrange("p (t q v) -> p t q v", t=ntg, v=ROW2)
                u2c = tree2(m24, q, ut1, ut2, ROW2)
                o2 = spool.tile([128, ntg * NCLS], f32, tag="o2")
                nc.vector.tensor_tensor(
                    o2[:].rearrange("p (t c) -> p t c", t=ntg),
                    u2c.squeeze(2)[:, :, 0:NCLS],
                    r2[:].unsqueeze(2).broadcast_to([128, ntg, NCLS]),
                    OP.mult,
                )
                nc.vector.tensor_tensor(
                    o2[:].rearrange("p (t c) -> p t c", t=ntg),
                    o2[:].rearrange("p (t c) -> p t c", t=ntg),
                    b2_sb[:, :].unsqueeze(1).broadcast_to([128, ntg, NCLS]),
                    OP.add,
                )
                # log_softmax
                mx = spool.tile([128, ntg], f32, tag="mx")
                nc.vector.tensor_reduce(
                    mx[:].unsqueeze(2),
                    o2[:].rearrange("p (t c) -> p t c", t=ntg),
                    axis=AX,
                    op=OP.max,
                )
                sh = spool.tile([128, ntg * NCLS], f32, tag="sh")
                nc.vector.tensor_tensor(
                    sh[:].rearrange("p (t c) -> p t c", t=ntg),
                    o2[:].rearrange("p (t c) -> p t c", t=ntg),
                    mx[:].unsqueeze(2).broadcast_to([128, ntg, NCLS]),
                    OP.subtract,
                )
                exs = spool.tile([128, ntg * NCLS], f32, tag="exs")
                nc.scalar.activation(exs[:], sh[:], AF.Exp)
                se = spool.tile([128, ntg], f32, tag="se")
                nc.vector.tensor_reduce(
                    se[:].unsqueeze(2),
                    exs[:].rearrange("p (t c) -> p t c", t=ntg),
                    axis=AX,
                    op=OP.add,
                )
                lg = spool.tile([128, ntg], f32, tag="lg")
                nc.scalar.activation(lg[:], se[:], AF.Ln)
                fin = spool.tile([128, ntg * NCLS], f32, tag="fin")
                nc.vector.tensor_tensor(
                    fin[:].rearrange("p (t c) -> p t c", t=ntg),
                    sh[:].rearrange("p (t c) -> p t c", t=ntg),
                    lg[:].unsqueeze(2).broadcast_to([128, ntg, NCLS]),
                    OP.subtract,
                )
                for ti, t in enumerate(tl):
                    P = min(128, NSH - t * 128)
                    nc.sync.dma_start(
                        out[t * 128 : t * 128 + P, :],
                        fin[0:P, ti * NCLS : (ti + 1) * NCLS],
                    )

    nc.compile()
    return nc


# --------------------------------------------------------------------------
_last_results = None


def kernel(**inputs):
    global _last_results
    import os

    meta, in_maps, perm_nodes = host_prep(inputs)
    nc = build_program(meta)
    from concourse import bass_utils

    trace = os.environ.get("GAT_TRACE") == "1"
    res = bass_utils.run_bass_kernel_spmd(
        nc, in_maps, core_ids=list(range(NC)), trace=trace
    )
    _last_results = res
    out_full = np.empty((N, NCLS), dtype=np.float32)
    for c in range(NC):
        out_full[perm_nodes[c]] = res.results[c]["out"]
    return out_full


# revision 13
# speedup vs baseline: 1.0089x; 1.0089x over previous
"""GAT (2-layer, CORA-style) on 8 Trainium2 NeuronCores.

Strategy: nodes dst-sharded across 8 cores (profile-sorted for padding).
  Phase 1: h = x@W1 in bf16 (+ attention scores), packed fp16 per-node
           table rows [h(64) | a_src(8)] -> DRAM, AllGather, then a
           restride DMA lays the table out at 256B row stride in 4
           windows of 25000 rows (+1 leading sentinel row per window)
           so gather indices fit int16.
  Edge phases: per group of 2 tiles (256 dsts), batched dma_gather ops
           (one per tile x window, ~2-3K rows each) fetch 144B rows at
           256B stride; segment-softmax with contiguous tree reductions;
           per-dst slot columns are padded per-group-uniform via
           sentinel rows (score -60 -> exp ~ 0).
  Phase 2: tiny fused matmul [W2 | W2@as2 | W2@ad2], second AllGather +
           restride (16B rows at 256B stride), second edge phase reusing
           the SAME index lists, log_softmax, output shard.
"""

import sys

sys.path.insert(0, "/opt/trn_rl_repo")

import numpy as np
import ml_dtypes

BF16 = ml_dtypes.bfloat16

N, E, F_IN = 100000, 3200000, 1433
H1, HID, NCLS = 8, 8, 7
NC = 8
NSH = N // NC
NW = 4
WNODES = 25000  # nodes per window
WROWS = WNODES + 1  # +1 sentinel row at window start
NROWT = NW * WROWS  # 100004
ROW1 = H1 * HID + H1  # 72
ROW2 = NCLS + 1  # 8
C1 = H1 * HID  # 64
VE = 128  # table row stride in f16 elements (256B)
NTG = 2  # tiles per group
RING = 1024  # max idxs per gather op (SWDGE ring capacity)
SCRATCH = 65536


def _win_row(r):
    # permuted-global node id -> strided table row index
    return (r // WNODES) * WROWS + 1 + (r % WNODES)


# --------------------------------------------------------------------------
def host_prep(inputs):
    x = np.asarray(inputs["x"], dtype=np.float32)
    ei = np.asarray(inputs["edge_index"], dtype=np.int64)
    W1 = np.asarray(inputs["W1"], dtype=np.float32)
    as1 = np.asarray(inputs["att_src1"], dtype=np.float32)
    ad1 = np.asarray(inputs["att_dst1"], dtype=np.float32)
    b1 = np.asarray(inputs["b1"], dtype=np.float32)
    W2 = np.asarray(inputs["W2"], dtype=np.float32)
    as2 = np.asarray(inputs["att_src2"], dtype=np.float32)
    ad2 = np.asarray(inputs["att_dst2"], dtype=np.float32)
    b2 = np.asarray(inputs["b2"], dtype=np.float32)

    loops = np.arange(N, dtype=np.int64)
    src_all = np.concatenate([ei[0], loops])
    dst_all = np.concatenate([ei[1], loops])

    # per-dst per-window counts of srcs (window by ORIGINAL src id is wrong:
    # windows are over PERMUTED ids; permutation is per-core independent, so
    # compute profiles over a provisional id = original (windows align with
    # final perm closely enough for sorting; exact cols computed later).
    # -> To keep it exact, do a 2-pass: first a degree-based permutation of
    # SRC ids is identity (srcs use the same gid map as dsts). We build the
    # permutation from profiles over original src windows, then recompute
    # everything with the actual gid.
    prof0 = np.zeros((N, NW), np.int32)
    np.add.at(prof0, (dst_all, np.minimum(src_all // WNODES, NW - 1)), 1)

    gid = np.empty(N, dtype=np.int64)
    perm_nodes = []
    for c in range(NC):
        lo = c * NSH
        p = prof0[lo : lo + NSH]
        # sort by the padding objective: a tile's cost is max over its rows
        # of max-per-window count; group rows with similar maxima together
        mw = p.max(axis=1)
        order = np.lexsort((-p.sum(axis=1), -mw))
        perm_nodes.append(lo + order)
        pos = np.empty(NSH, dtype=np.int64)
        pos[order] = np.arange(NSH)
        gid[lo : lo + NSH] = lo + pos

    pd = gid[dst_all]
    gs = gid[src_all].astype(np.int64)
    eorder = np.argsort(pd, kind="stable")
    gss = gs[eorder]
    counts = np.bincount(pd, minlength=N)
    row_start = np.zeros(N + 1, np.int64)
    np.cumsum(counts, out=row_start[1:])

    NT = (NSH + 127) // 128
    # per (core, tile-row p, window w) sorted src lists
    # cols[t][w] = max over cores,rows of per-window count
    cols = np.zeros((NT, NW), np.int64)
    wof_s = gss // WNODES  # window of each (dst-sorted) edge's src
    for c in range(NC):
        base = c * NSH
        cw = np.zeros((NSH, NW), np.int32)
        np.add.at(
            cw,
            (
                np.repeat(np.arange(NSH), np.diff(row_start[base : base + NSH + 1])),
                wof_s[row_start[base] : row_start[base + NSH]],
            ),
            1,
        )
        for t in range(NT):
            blk = cw[t * 128 : (t + 1) * 128]
            cols[t] = np.maximum(cols[t], blk.max(axis=0))

    # groups of NTG tiles with uniform per-(t,w) col count C_g (even)
    groups = []
    for g0 in range(0, NT, NTG):
        tl = list(range(g0, min(g0 + NTG, NT)))
        C = int(max(cols[t].max() for t in tl))
        C = max(2, C + (C & 1))
        groups.append(dict(tiles=tl, C=C))

    # build idx lists (shared schedule; per-core values)
    # G layout per group: [tile][window][C] columns, each column = 128 slots
    # op = (tile, window, col-subrange) with ni = 128*ncols <= RING
    idx_parts = [[] for _ in range(NC)]
    op_sched = []  # per group: list of (gcol0, ncols, w, ni, idx_off)
    idx_off = 0
    for gi, grp in enumerate(groups):
        C = grp["C"]
        ops = []
        for ti, t in enumerate(grp["tiles"]):
            for w in range(NW):
                ncols_tot = C
                csub0 = 0
                while csub0 < ncols_tot:
                    ncols = min(ncols_tot - csub0, RING // 128)
                    ni = 128 * ncols
                    gcol0 = ti * (NW * C) + w * C + csub0
                    ops.append((gcol0, ncols, w, ni, idx_off))
                    idx_off += 128 * (ni // 16)  # int16 elements (wrapped layout)
                    csub0 += ncols
        grp["ops"] = ops
        op_sched.append(ops)
    TOTIDX = idx_off

    # fill per-core idx data
    for c in range(NC):
        base = c * NSH
        # per row: sorted srcs split by window (window-local strided row ids)
        row_lists = []
        for p in range(NSH):
            seg = np.sort(gss[row_start[base + p] : row_start[base + p + 1]])
            wsplit = []
            for w in range(NW):
                lo = np.searchsorted(seg, w * WNODES)
                hi = np.searchsorted(seg, (w + 1) * WNODES)
                wsplit.append(1 + (seg[lo:hi] - w * WNODES))
            row_lists.append(wsplit)
        arr = np.zeros(TOTIDX, dtype=np.int16)
        for gi, grp in enumerate(groups):
            C = grp["C"]
            for gcol0, ncols, w, ni, off in grp["ops"]:
                ti = gcol0 // (NW * C)
                csub0 = gcol0 - ti * (NW * C) - w * C
                t = grp["tiles"][ti]
                # idx list: column-major: i = (c_loc*128 + p)
                blk = np.zeros((ncols, 128), dtype=np.int16)
                for p in range(min(128, NSH - t * 128)):
                    lst = row_lists[t * 128 + p][w]
                    lo = csub0
                    hi = min(len(lst), csub0 + ncols)
                    if hi > lo:
                        blk[0 : hi - lo, p] = lst[lo:hi]
                flat = blk.reshape(-1)  # i = c_loc*128+p
                # wrapped [128, ni/16]: entry i at [i%16, i//16], replicated x8
                wrap = np.zeros((128, ni // 16), dtype=np.int16)
                w16 = flat.reshape(ni // 16, 16).T
                for rg in range(8):
                    wrap[rg * 16 : (rg + 1) * 16, :] = w16
                arr[off : off + 128 * (ni // 16)] = wrap.reshape(-1)
        idx_parts[c] = arr

    # x shards (bf16, transposed)
    xts = [np.ascontiguousarray(x[perm_nodes[c]].T).astype(BF16) for c in range(NC)]

    attm = np.zeros((C1, 2 * H1), dtype=np.float32)
    for h in range(H1):
        attm[h * HID : (h + 1) * HID, h] = as1[h]
        attm[h * HID : (h + 1) * HID, H1 + h] = ad1[h]
    m2 = np.zeros((HID, NCLS + 2), dtype=np.float32)
    m2[:, :NCLS] = W2
    m2[:, NCLS] = W2 @ as2[0]
    m2[:, NCLS + 1] = W2 @ ad2[0]
    b1r = np.tile(b1[None, :], (128, 1)).astype(np.float32)
    b2r = np.tile(b2[None, :], (128, 1)).astype(np.float32)
    # sentinel rows (one per window, at strided row w*WROWS)
    sent1 = np.zeros((NW, ROW1), dtype=np.float16)
    sent1[:, C1:] = -60.0
    sent2 = np.zeros((NW, ROW2), dtype=np.float16)
    sent2[:, NCLS] = -60.0
    ident = np.eye(128, dtype=np.float32)

    meta = dict(NT=NT, groups=groups, TOTIDX=TOTIDX)
    in_maps = []
    for c in range(NC):
        in_maps.append(
            dict(
                xt=xts[c],
                w1=W1.astype(BF16),
                attm=attm,
                m2=m2.astype(BF16),
                b1r=b1r,
                b2r=b2r,
                sent1=sent1,
                sent2=sent2,
                ident=ident,
                idx=idx_parts[c],
            )
        )
    return meta, in_maps, perm_nodes


# --------------------------------------------------------------------------
def build_program(meta):
    from concourse import bacc, bass, tile, mybir
    from concourse.mybir import InstDMAGatherAnt

    NT = meta["NT"]
    groups = meta["groups"]
    TOTIDX = meta["TOTIDX"]
    f32, f16, i32 = mybir.dt.float32, mybir.dt.float16, mybir.dt.int32
    i16, bf16 = mybir.dt.int16, mybir.dt.bfloat16
    AX = mybir.AxisListType.X
    OP = mybir.AluOpType
    AF = mybir.ActivationFunctionType

    nc = bacc.Bacc(
        "TRN2",
        target_bir_lowering=False,
        debug=False,
        num_devices=NC,
        dynamic_dma_scratch_size=SCRATCH,
        num_swdge_queues=4,
    )

    xt = nc.dram_tensor("xt", [F_IN, NSH], bf16, kind="ExternalInput").ap()
    w1 = nc.dram_tensor("w1", [F_IN, C1], bf16, kind="ExternalInput").ap()
    attm = nc.dram_tensor("attm", [C1, 2 * H1], f32, kind="ExternalInput").ap()
    m2 = nc.dram_tensor("m2", [HID, NCLS + 2], bf16, kind="ExternalInput").ap()
    b1r = nc.dram_tensor("b1r", [128, HID], f32, kind="ExternalInput").ap()
    b2r = nc.dram_tensor("b2r", [128, NCLS], f32, kind="ExternalInput").ap()
    sent1 = nc.dram_tensor("sent1", [NW, ROW1], f16, kind="ExternalInput").ap()
    sent2 = nc.dram_tensor("sent2", [NW, ROW2], f16, kind="ExternalInput").ap()
    ident = nc.dram_tensor("ident", [128, 128], f32, kind="ExternalInput").ap()
    idx_h = nc.dram_tensor("idx", [TOTIDX], i16, kind="ExternalInput")
    out = nc.dram_tensor("out", [NSH, NCLS], f32, kind="ExternalOutput").ap()

    ktl = []
    k0 = 0
    while k0 < F_IN:
        ktl.append((k0, min(128, F_IN - k0)))
        k0 += 128
    chunks = []
    n0 = 0
    while n0 < NSH:
        chunks.append((n0, min(512, NSH - n0)))
        n0 += 512

    def slices_of(c0, csz):
        s0, outl = 0, []
        while s0 < csz:
            ssz = min(128, csz - s0)
            outl.append((c0 + s0, ssz))
            s0 += ssz
        return outl

    def raw_gather(out_ap, in_ap, idxs_ap, num_idxs, elem_size, qn=0):
        eng = nc.gpsimd
        elem_step = in_ap.ap[0][0]
        stride_bytes = elem_step * mybir.dt.size(in_ap.dtype)
        return eng.add_instruction(
            InstDMAGatherAnt(
                name=f"I-{eng.bass.next_id()}",
                ins=[
                    *eng.lower_ap_dma(in_ap, for_custom_bir_dma=True),
                    eng.lower_ap(idxs_ap),
                    eng.lower_val_access(eng.to_reg(num_idxs)),
                ],
                outs=[eng.lower_ap(out_ap)],
                transpose=False,
                num_idxs=num_idxs,
                elem_size=elem_size,
                stride_bytes_256=stride_bytes // 256,
                gen_mode=0,
                single_packet=True,
                queue_num=qn,
                sbuf_tokens_per_rank=0,
                sbuf_free_dim_per_rank=0,
                sbuf_free_dim_pad_per_rank=0,
                sbuf_byte_offset=0,
            )
        )

    with tile.TileContext(nc) as tc:
        with (
            tc.tile_pool(name="dram", bufs=1, space="DRAM") as dpool,
            tc.tile_pool(name="consts", bufs=1) as cpool,
            tc.tile_pool(name="persist", bufs=1) as ppool,
            tc.tile_pool(name="xload", bufs=4) as xpool,
            tc.tile_pool(name="work", bufs=2) as wpool,
            tc.tile_pool(name="scr", bufs=1) as spool,
            tc.tile_pool(name="ps", bufs=2, space="PSUM") as pspool,
        ):
            tb1s = dpool.tile([NSH, ROW1], f16, tag="tb1s")
            tb1c = dpool.tile([N, ROW1], f16, tag="tb1c")
            tb1f = dpool.tile([NROWT, VE], f16, tag="tb1f")
            tb2s = dpool.tile([NSH, ROW2], f16, tag="tb2s")
            tb2c = dpool.tile([N, ROW2], f16, tag="tb2c")
            tb2f = dpool.tile([NROWT, VE], f16, tag="tb2f")

            w1t = []
            for ki, (k0, ks) in enumerate(ktl):
                wt = cpool.tile([ks, C1], bf16, tag=f"w1_{ki}")
                nc.sync.dma_start(wt[:], w1[k0 : k0 + ks, :])
                w1t.append(wt)
            attm_sb = cpool.tile([C1, 2 * H1], f32, tag="attm")
            nc.sync.dma_start(attm_sb[:], attm[:])
            m2_sb = cpool.tile([HID, NCLS + 2], bf16, tag="m2")
            nc.sync.dma_start(m2_sb[:], m2[:])
            b1_sb = cpool.tile([128, HID], f32, tag="b1")
            nc.sync.dma_start(b1_sb[:], b1r[:])
            b2_sb = cpool.tile([128, NCLS], f32, tag="b2")
            nc.sync.dma_start(b2_sb[:], b2r[:])
            id_sb = cpool.tile([128, 128], f32, tag="ident")
            nc.sync.dma_start(id_sb[:], ident[:])
            s1_sb = cpool.tile([NW, ROW1], f16, tag="s1")
            nc.sync.dma_start(s1_sb[:], sent1[:])
            s2_sb = cpool.tile([NW, ROW2], f16, tag="s2")
            nc.sync.dma_start(s2_sb[:], sent2[:])
            nbias = cpool.tile([128, 1], f32, tag="nbias")
            nc.gpsimd.memset(nbias[:], -8.0)
            # sentinel rows into strided tables (row w*WROWS)
            for w in range(NW):
                nc.sync.dma_start(
                    tb1f[w * WROWS : w * WROWS + 1, 0:ROW1], s1_sb[w : w + 1, :]
                )
                nc.sync.dma_start(
                    tb2f[w * WROWS : w * WROWS + 1, 0:ROW2], s2_sb[w : w + 1, :]
                )

            ad1_all = ppool.tile([128, NT * H1], f32, tag="ad1_all")
            ad2_all = ppool.tile([128, NT], f32, tag="ad2_all")
            eluT = ppool.tile([HID, NSH], bf16, tag="eluT")

            # =========================== PHASE 1 ===========================
            for ch, (c0, csz) in enumerate(chunks):
                hp = pspool.tile([C1, csz], f32, tag="mm")
                for ki, (k0, ks) in enumerate(ktl):
                    xtt = xpool.tile([ks, csz], bf16, tag="xt")
                    nc.sync.dma_start(xtt[:], xt[k0 : k0 + ks, c0 : c0 + csz])
                    nc.tensor.matmul(
                        hp[:],
                        w1t[ki][:],
                        xtt[:],
                        start=(ki == 0),
                        stop=(ki == len(ktl) - 1),
                    )
                h_sb = wpool.tile([C1, csz], f32, tag="h_sb")
                nc.vector.tensor_copy(h_sb[:], hp[:])
                sp = pspool.tile([2 * H1, csz], f32, tag="sc")
                nc.tensor.matmul(sp[:], attm_sb[:], h_sb[:], start=True, stop=True)
                sc_sb = wpool.tile([2 * H1, csz], f32, tag="sc_sb")
                nc.vector.tensor_copy(sc_sb[:], sp[:])
                for s0, ssz in slices_of(c0, csz):
                    t_idx = s0 // 128
                    sl = slice(s0 - c0, s0 - c0 + ssz)
                    tp = pspool.tile([ssz, C1], f32, tag="tr")
                    nc.tensor.transpose(tp[:], h_sb[:, sl], id_sb[:C1, :C1])
                    st = pspool.tile([ssz, 2 * H1], f32, tag="tr2")
                    nc.tensor.transpose(st[:], sc_sb[:, sl], id_sb[: 2 * H1, : 2 * H1])
                    row = wpool.tile([ssz, ROW1], f16, tag="row")
                    nc.scalar.activation(row[:, 0:C1], tp[:], AF.Copy)
                    nc.vector.tensor_copy(row[:, C1:ROW1], st[:, 0:H1])
                    nc.vector.tensor_copy(
                        ad1_all[0:ssz, t_idx * H1 : (t_idx + 1) * H1],
                        st[:, H1 : 2 * H1],
                    )
                    nc.sync.dma_start(tb1s[s0 : s0 + ssz, :], row[:])

            nc.gpsimd.collective_compute(
                "AllGather",
                OP.bypass,
                replica_groups=[list(range(NC))],
                ins=[tb1s[:].opt()],
                outs=[tb1c[:].opt()],
            )
            # restride into windowed 256B-stride layout
            for w in range(NW):
                nc.sync.dma_start(
                    tb1f[w * WROWS + 1 : w * WROWS + 1 + WNODES, 0:ROW1],
                    tb1c[w * WNODES : (w + 1) * WNODES, :],
                )

            # =========================== EDGE PHASE 1 ======================
            for gi, grp in enumerate(groups):
                C = grp["C"]
                tl = grp["tiles"]
                ntg = len(tl)
                S = ntg * NW * C  # total columns in group
                G = wpool.tile([128, S * ROW1], f16, tag="G")
                for opi, (gcol0, ncols, w, ni, off) in enumerate(grp["ops"]):
                    itb = wpool.tile([128, ni // 16], i16, tag="itb")
                    nc.sync.dma_start(
                        itb[:],
                        bass.AP(idx_h, off, [[ni // 16, 128], [1, ni // 16]]),
                    )
                    raw_gather(
                        G[:, gcol0 * ROW1 : (gcol0 + ncols) * ROW1].rearrange(
                            "p (j v) -> p j v", v=ROW1
                        ),
                        tb1f[w * WROWS : (w + 1) * WROWS, 0:ROW1],
                        itb[:],
                        ni,
                        ROW1,
                        qn=opi % 4,
                    )
                # scores: s = a_src (from G) + a_dst (local)
                G4 = G[:].rearrange("p (t q v) -> p t q v", t=ntg, v=ROW1)
                sT = spool.tile([128, S * H1], f32, tag="sT")
                ad_b = (
                    ad1_all[:, tl[0] * H1 : (tl[0] + ntg) * H1]
                    .rearrange("p (t h) -> p t h", t=ntg)
                    .unsqueeze(2)
                    .broadcast_to([128, ntg, NW * C, H1])
                )
                nc.vector.tensor_tensor(
                    sT[:].rearrange("p (t q h) -> p t q h", t=ntg, h=H1),
                    G4[:, :, :, C1:ROW1],
                    ad_b,
                    OP.add,
                )
                pT = spool.tile([128, S * H1], f16, tag="pT")
                nc.vector.tensor_scalar(pT[:], sT[:], 0.2, None, op0=OP.mult)
                nc.vector.tensor_tensor(sT[:], sT[:], pT[:], OP.max)
                nc.scalar.activation(pT[:], sT[:], AF.Exp, bias=nbias[:])
                # den tree: [128, ntg, NW*C, 8] halve axis 2
                q = NW * C
                dt1 = spool.tile([128, ntg * (q // 2) * H1], f16, tag="dt1")
                dt2 = spool.tile([128, ntg * (q // 4 + 1) * H1], f16, tag="dt2")

                def tree(src_ap, qq, t1, t2, width):
                    # src_ap viewed [128, ntg, qq, width]; returns ap [128, ntg, 1, width]
                    cur = src_ap
                    buf = [t1, t2]
                    bi = 0
                    while qq > 1:
                        half = qq // 2
                        odd = qq - 2 * half
                        dst = buf[bi][:].rearrange(
                            "p (t q w) -> p t q w", t=ntg, w=width
                        )[:, :, 0 : half + odd, :]
                        nc.vector.tensor_tensor(
                            dst[:, :, 0:half, :],
                            cur[:, :, 0:half, :],
                            cur[:, :, half : 2 * half, :],
                            OP.add,
                        )
                        if odd:
                            nc.vector.tensor_copy(
                                dst[:, :, half : half + 1, :],
                                cur[:, :, 2 * half : qq, :],
                            )
                        cur = dst
                        qq = half + odd
                        bi ^= 1
                    return cur

                pT4 = pT[:].rearrange("p (t q h) -> p t q h", t=ntg, h=H1)
                denc = tree(pT4, q, dt1, dt2, H1)
                den = spool.tile([128, ntg * H1], f32, tag="den")
                nc.vector.tensor_copy(
                    den[:].rearrange("p (t h) -> p t h", t=ntg),
                    denc.squeeze(2),
                )
                rr = spool.tile([128, ntg * H1], f32, tag="rr")
                nc.vector.reciprocal(rr[:], den[:])
                nc.vector.tensor_scalar(rr[:], rr[:], 1.0 / H1, None, op0=OP.mult)
                # messages m1 = h * p (in-place into G's h columns)
                Ghv = (
                    G[:]
                    .rearrange("p (s v) -> p s v", v=ROW1)[:, :, 0:C1]
                    .rearrange("p s (h c) -> p s h c", h=H1)
                )
                nc.vector.tensor_tensor(
                    Ghv,
                    Ghv,
                    pT[:]
                    .rearrange("p (s h) -> p s h", h=H1)
                    .unsqueeze(3)
                    .broadcast_to([128, S, H1, HID]),
                    OP.mult,
                )
                mt1 = spool.tile([128, ntg * (q // 2) * C1], f16, tag="mt1")
                mt2 = spool.tile([128, ntg * (q // 4 + 1) * C1], f16, tag="mt2")
                m14 = G[:].rearrange("p (t q v) -> p t q v", t=ntg, v=ROW1)[
                    :, :, :, 0:C1
                ]
                uc = tree(m14, q, mt1, mt2, C1)
                # o1 = u * rr (broadcast over c), then mean over h, +b1, elu
                o1 = spool.tile([128, ntg * C1], f32, tag="o1")
                nc.vector.tensor_tensor(
                    o1[:].rearrange("p (t h c) -> p t h c", t=ntg, c=HID),
                    uc.squeeze(2).rearrange("p t (h c) -> p t h c", c=HID),
                    rr[:]
                    .rearrange("p (t h) -> p t h", t=ntg)
                    .unsqueeze(3)
                    .broadcast_to([128, ntg, H1, HID]),
                    OP.mult,
                )
                om = spool.tile([128, ntg * HID], f32, tag="om")
                nc.vector.tensor_reduce(
                    om[:].rearrange("p (t c) -> p t c", t=ntg),
                    o1[:].rearrange("p (t h c) -> p t c h", t=ntg, c=HID),
                    axis=AX,
                    op=OP.add,
                )
                nc.vector.tensor_tensor(
                    om[:].rearrange("p (t c) -> p t c", t=ntg),
                    om[:].rearrange("p (t c) -> p t c", t=ntg),
                    b1_sb[:, :].unsqueeze(1).broadcast_to([128, ntg, HID]),
                    OP.add,
                )
                mn = spool.tile([128, ntg * HID], f32, tag="mn")
                nc.vector.tensor_scalar(mn[:], om[:], 0.0, None, op0=OP.min)
                nc.scalar.activation(mn[:], mn[:], AF.Exp)
                rl = spool.tile([128, ntg * HID], f32, tag="rl")
                nc.vector.tensor_scalar(rl[:], om[:], 0.0, None, op0=OP.max)
                nc.vector.tensor_scalar(mn[:], mn[:], -1.0, None, op0=OP.add)
                nc.vector.tensor_tensor(rl[:], rl[:], mn[:], OP.add)
                for ti, t in enumerate(tl):
                    P = min(128, NSH - t * 128)
                    ep = pspool.tile([HID, P], f32, tag="sc")
                    nc.tensor.transpose(
                        ep[:], rl[0:P, ti * HID : (ti + 1) * HID], id_sb[:P, :P]
                    )
                    nc.scalar.activation(
                        eluT[:, t * 128 : t * 128 + P], ep[:], AF.Copy
                    )

            # =========================== PHASE 2 ===========================
            for ch, (c0, csz) in enumerate(chunks):
                o2p = pspool.tile([NCLS + 2, csz], f32, tag="mm")
                nc.tensor.matmul(
                    o2p[:], m2_sb[:], eluT[:, c0 : c0 + csz], start=True, stop=True
                )
                o2_sb = wpool.tile([NCLS + 2, csz], f32, tag="o2_sb")
                nc.vector.tensor_copy(o2_sb[:], o2p[:])
                for s0, ssz in slices_of(c0, csz):
                    t_idx = s0 // 128
                    sl = slice(s0 - c0, s0 - c0 + ssz)
                    tp2 = pspool.tile([ssz, NCLS + 2], f32, tag="tr")
                    nc.tensor.transpose(
                        tp2[:], o2_sb[:, sl], id_sb[: NCLS + 2, : NCLS + 2]
                    )
                    row2 = wpool.tile([ssz, ROW2], f16, tag="row2")
                    nc.scalar.activation(row2[:], tp2[:, 0:ROW2], AF.Copy)
                    nc.vector.tensor_copy(
                        ad2_all[0:ssz, t_idx : t_idx + 1], tp2[:, ROW2 : ROW2 + 1]
                    )
                    nc.sync.dma_start(tb2s[s0 : s0 + ssz, :], row2[:])

            nc.gpsimd.collective_compute(
                "AllGather",
                OP.bypass,
                replica_groups=[list(range(NC))],
                ins=[tb2s[:].opt()],
                outs=[tb2c[:].opt()],
            )
            for w in range(NW):
                nc.sync.dma_start(
                    tb2f[w * WROWS + 1 : w * WROWS + 1 + WNODES, 0:ROW2],
                    tb2c[w * WNODES : (w + 1) * WNODES, :],
                )

            # =========================== EDGE PHASE 2 ======================
            for gi, grp in enumerate(groups):
                C = grp["C"]
                tl = grp["tiles"]
                ntg = len(tl)
                S = ntg * NW * C
                q = NW * C
                G2 = wpool.tile([128, S * ROW2], f16, tag="G2")
                for opi, (gcol0, ncols, w, ni, off) in enumerate(grp["ops"]):
                    itb = wpool.tile([128, ni // 16], i16, tag="itb")
                    nc.sync.dma_start(
                        itb[:],
                        bass.AP(idx_h, off, [[ni // 16, 128], [1, ni // 16]]),
                    )
                    raw_gather(
                        G2[:, gcol0 * ROW2 : (gcol0 + ncols) * ROW2].rearrange(
                            "p (j v) -> p j v", v=ROW2
                        ),
                        tb2f[w * WROWS : (w + 1) * WROWS, 0:ROW2],
                        itb[:],
                        ni,
                        ROW2,
                        qn=opi % 4,
                    )
                s2 = spool.tile([128, S], f32, tag="s2")
                nc.vector.tensor_tensor(
                    s2[:].rearrange("p (t q) -> p t q", t=ntg),
                    G2[:]
                    .rearrange("p (t q v) -> p t q v", t=ntg, v=ROW2)[:, :, :, NCLS]
                    .squeeze(),
                    ad2_all[:, tl[0] : tl[0] + ntg]
                    .unsqueeze(2)
                    .broadcast_to([128, ntg, q]),
                    OP.add,
                )
                p2 = spool.tile([128, S], f16, tag="p2")
                nc.vector.tensor_scalar(p2[:], s2[:], 0.2, None, op0=OP.mult)
                nc.vector.tensor_tensor(s2[:], s2[:], p2[:], OP.max)
                nc.scalar.activation(p2[:], s2[:], AF.Exp)
                et1 = spool.tile([128, ntg * (q // 2)], f16, tag="et1")
                et2 = spool.tile([128, ntg * (q // 4 + 1)], f16, tag="et2")

                def tree2(src_ap, qq, t1, t2, width):
                    cur = src_ap
                    buf = [t1, t2]
                    bi = 0
                    while qq > 1:
                        half = qq // 2
                        odd = qq - 2 * half
                        dst = buf[bi][:].rearrange(
                            "p (t q w) -> p t q w", t=ntg, w=width
                        )[:, :, 0 : half + odd, :]
                        nc.vector.tensor_tensor(
                            dst[:, :, 0:half, :],
                            cur[:, :, 0:half, :],
                            cur[:, :, half : 2 * half, :],
                            OP.add,
                        )
                        if odd:
                            nc.vector.tensor_copy(
                                dst[:, :, half : half + 1, :],
                                cur[:, :, 2 * half : qq, :],
                            )
                        cur = dst
                        qq = half + odd
                        bi ^= 1
                    return cur

                p24 = p2[:].rearrange("p (t q) -> p t q", t=ntg).unsqueeze(3)
                d2c = tree2(p24, q, et1, et2, 1)
                den2 = spool.tile([128, ntg], f32, tag="den2")
                nc.vector.tensor_copy(den2[:].unsqueeze(2).unsqueeze(3), d2c)
                r2 = spool.tile([128, ntg], f32, tag="r2")
                nc.vector.reciprocal(r2[:], den2[:])
                G2v = G2[:].rearrange("p (s v) -> p s v", v=ROW2)
                nc.vector.tensor_tensor(
                    G2v,
                    G2v,
                    p2[:].unsqueeze(2).broadcast_to([128, S, ROW2]),
                    OP.mult,
                )
                ut1 = spool.tile([128, ntg * (q // 2) * ROW2], f16, tag="ut1")
                ut2 = spool.tile([128, ntg * (q // 4 + 1) * ROW2], f16, tag="ut2")
                m24 = G2[:].rearrange("p (t q v) -> p t q v", t=ntg, v=ROW2)
                u2c = tree2(m24, q, ut1, ut2, ROW2)
                o2 = spool.tile([128, ntg * NCLS], f32, tag="o2")
                nc.vector.tensor_tensor(
                    o2[:].rearrange("p (t c) -> p t c", t=ntg),
                    u2c.squeeze(2)[:, :, 0:NCLS],
                    r2[:].unsqueeze(2).broadcast_to([128, ntg, NCLS]),
                    OP.mult,
                )
                nc.vector.tensor_tensor(
                    o2[:].rearrange("p (t c) -> p t c", t=ntg),
                    o2[:].rearrange("p (t c) -> p t c", t=ntg),
                    b2_sb[:, :].unsqueeze(1).broadcast_to([128, ntg, NCLS]),
                    OP.add,
                )
                # log_softmax
                mx = spool.tile([128, ntg], f32, tag="mx")
                nc.vector.tensor_reduce(
                    mx[:].unsqueeze(2),
                    o2[:].rearrange("p (t c) -> p t c", t=ntg),
                    axis=AX,
                    op=OP.max,
                )
                sh = spool.tile([128, ntg * NCLS], f32, tag="sh")
                nc.vector.tensor_tensor(
                    sh[:].rearrange("p (t c) -> p t c", t=ntg),
                    o2[:].rearrange("p (t c) -> p t c", t=ntg),
                    mx[:].unsqueeze(2).broadcast_to([128, ntg, NCLS]),
                    OP.subtract,
                )
                exs = spool.tile([128, ntg * NCLS], f32, tag="exs")
                nc.scalar.activation(exs[:], sh[:], AF.Exp)
                se = spool.tile([128, ntg], f32, tag="se")
                nc.vector.tensor_reduce(
                    se[:].unsqueeze(2),
                    exs[:].rearrange("p (t c) -> p t c", t=ntg),
                    axis=AX,
                    op=OP.add,
                )
                lg = spool.tile([128, ntg], f32, tag="lg")
                nc.scalar.activation(lg[:], se[:], AF.Ln)
                fin = spool.tile([128, ntg * NCLS], f32, tag="fin")
                nc.vector.tensor_tensor(
                    fin[:].rearrange("p (t c) -> p t c", t=ntg),
                    sh[:].rearrange("p (t c) -> p t c", t=ntg),
                    lg[:].unsqueeze(2).broadcast_to([128, ntg, NCLS]),
                    OP.subtract,
                )
                for ti, t in enumerate(tl):
                    P = min(128, NSH - t * 128)
                    nc.sync.dma_start(
                        out[t * 128 : t * 128 + P, :],
                        fin[0:P, ti * NCLS : (ti + 1) * NCLS],
                    )

    nc.compile()
    return nc


# --------------------------------------------------------------------------
_last_results = None


def kernel(**inputs):
    global _last_results
    import os

    meta, in_maps, perm_nodes = host_prep(inputs)
    nc = build_program(meta)
    from concourse import bass_utils

    trace = os.environ.get("GAT_TRACE") == "1"
    res = bass_utils.run_bass_kernel_spmd(
        nc, in_maps, core_ids=list(range(NC)), trace=trace
    )
    _last_results = res
    out_full = np.empty((N, NCLS), dtype=np.float32)
    for c in range(NC):
        out_full[perm_nodes[c]] = res.results[c]["out"]
    return out_full
